# revision 11
# baseline (speedup 1.0000x reference)
"""Trainium2 Bass kernel for nn_Bdfdv_51170240364850 (gnn_message_passing).

Computes, for mode pairs (il, im) with im <= il (L1 = 5 modes each way) and
spatial/velocity grid (nx=1024, nv=512):

  D[il,im] = base + (-1j)*im*bx*F[il,im] + cB*bm*F[il,im+1]
             + [im==0] Re(cC*bp*F[il,1])
  base     = 0.5*bm*F[il,im-1]  (il>=1, 1<=im<=il)   else  D0[il,im]

with bx = b[:,0], bm = b[:,1]+1j b[:,2], bp = conj(bm),
cB = -(il-im)(il+im+1)/2, cC = -il(il+1).

Strategy: pure data-parallel over nx across 8 NeuronCores (nx=128 per core on
the 128 SBUF partitions), fp16 I/O.  Every per-x product c(x)*T runs as a
diagonal-weight matmul accumulating in PSUM (diag(c) @ tile scales partition
row p by c(p)).  The 13 diagonal weight tiles are built ON-CHIP from a tiny
12-column scalar table (identity via affine_select, then one tensor_scalar
per diagonal), so the input stream is only the F/D0 payload (4.72 MB/core).

Pipeline: inputs stream in arrival order run1, run0, run2, run3, run4, D0i,
D0r on the sync HWDGE ring; the PE program is emitted in the same order so
each pair's 6-matmul chain closes right behind its data.  b0 (im=0) partial
sums live in a separate pinned PSUM pool so their long-lived banks (waiting
on the late D0r input) never block the rotating mid/diag pair banks --
the serialization that stalled the previous version.  Dense 512-wide warmup
matmuls at the head keep the HAM activity window busy so real chains run at
the warm 2.4 GHz clock.  Evacuations split ACT (early pairs) / DVE (late
pairs); outputs leave as 7 run-sized DMAs on the scalar HWDGE ring (early)
and the sync ring behind the inputs (late), so input+output interleave and
the 16 SDMA engines stay saturated.
"""

import numpy as np

import bass_rust
import concourse.bass as bass
import concourse.tile as tile
from concourse import mybir
from concourse.bass_utils import run_bass_kernel_spmd

L1 = 5
NX = 1024
NV = 512
NCORES = 8
XS = NX // NCORES  # 128, = SBUF partitions

F32 = mybir.dt.float32
F16 = mybir.dt.float16

# ---------------------------------------------------------------------------
# run/slot bookkeeping.  Run m holds slots (m, il) for il = ILMIN[m]..4,
# laid out [re slots | im slots] back-to-back so each run is ONE contiguous
# DMA.  Runs are stored in STREAM order run1, run0, run2, run3, run4.
RL = {0: 4, 1: 4, 2: 3, 3: 2, 4: 1}          # run lengths
ILMIN = {m: max(1, m) for m in range(L1)}
STREAM = [1, 0, 2, 3, 4]
FOFF = {}
_o = 0
for _m in STREAM:
    FOFF[_m] = _o
    _o += 2 * RL[_m]
assert _o == 28                               # NV-column units

# output block offsets (NV units): runs 1..4 then the im=0 block [Dr0|Di0]
RO = {1: 0, 2: 8, 3: 14, 4: 18}
RO0 = 20                                       # Dr0 at 20..24, Di0 at 24..28
COUT = 28 * NV

# input pin layout (NV units): F runs (28) | D0i (4) | D0r (4)
D0I_OFF = 28
D0R_OFF = 32
CIN = 36 * NV

# per-(il,im) recurrence pairs, in pipeline order (g=1 mids, then g=2, g=3)
CB_PAIRS = [(2, 1), (3, 1), (4, 1), (3, 2), (4, 2), (4, 3)]  # (il, im)


def _cB(il, im):
    return -(il - im) * (il + im + 1) / 2.0


# diagonal-weight table columns (pscal) and W tile indices
DG_D1, DG_D2, DG_D3 = 0, 1, 2                 # 0.5b1, 0.5b2, -0.5b2
DG_D6 = 11                                    # 1.5b1
DG_ONES = 12                                  # identity (built on-chip)
NSCAL = 16                                    # 12 used + padding


def DG_AP(m):
    return 2 + m          # 3..6:  +m*b0


def DG_AN(m):
    return 6 + m          # 7..10: -m*b0


# ---------------------------------------------------------------------------
# The walrus build in this container rejects instructions carrying more than
# ONE sync-wait ("Too many sync wait commands", setupSyncWait in
# CoreV2/V3GenImpl). Tile's scheduler routinely attaches several. Post-pass:
# hoist all but the last wait of each instruction onto same-engine NOPs
# inserted immediately before it (same basic block, so per-engine program
# order is preserved).
def split_multiwaits(nc):
    for f in nc.m.functions:
        for blk in f.blocks:
            new = []
            changed = False
            for ins in blk.instructions:
                si = ins.sync_info
                if si is not None and len(si.on_wait) > 1:
                    waits = list(si.on_wait)
                    for w in waits[:-1]:
                        nop = mybir.InstNoOp(
                            name=nc.get_next_instruction_name(),
                            engine=ins.engine,
                            bass_nofuse=True,
                            sync_info=mybir.SyncInfo(on_wait=[w],
                                                     on_update=[]),
                        )
                        new.append(nop)
                    ins.sync_info = bass_rust.SyncInfo(
                        on_wait=[waits[-1]], on_update=list(si.on_update))
                    changed = True
                new.append(ins)
            if changed:
                blk.instructions = new


# ---------------------------------------------------------------------------
def _pair(ap, step_elems, nblocks=2):
    """Turn a contiguous [P, L] AP into [P, nblocks, L] with the given
    element step between blocks."""
    c = ap.copy()
    v = c.ap
    last = v.pop()
    v.append((step_elems, nblocks))
    v.append(tuple(last))
    c.ap = v
    return c


NWARM = 10


def build_bass(split=True):
    MULT = mybir.AluOpType.mult
    ADD = mybir.AluOpType.add

    nc = bass.Bass()
    pin = nc.dram_tensor("pin", [XS, CIN], F16, kind="ExternalInput").ap()
    pscal = nc.dram_tensor("pscal", [XS, NSCAL], F32,
                           kind="ExternalInput").ap()
    pout = nc.dram_tensor("pout", [XS, COUT], F16, kind="ExternalOutput").ap()

    with tile.TileContext(nc) as tc:
        with tc.tile_pool(name="m", bufs=1) as pool, \
             tc.psum_pool(name="pp", bufs=2) as ppool, \
             tc.psum_pool(name="pb", bufs=2) as bpool:
            fF = pool.tile([XS, 28 * NV], F16, tag="fF")
            fD0 = pool.tile([XS, 8 * NV], F16, tag="fD0")
            scal = pool.tile([XS, NSCAL], F32, tag="scal")
            fW = pool.tile([XS, 13 * 128], F16, tag="fW")
            ones128 = pool.tile([XS, 128], F16, tag="ones")
            wrhs = pool.tile([XS, NV], F16, tag="wrhs")
            P = pool.tile([XS, 2 * 6 * NV], F16, tag="P")
            G = pool.tile([XS, 2 * 4 * NV], F16, tag="G")
            OUT = pool.tile([XS, 28 * NV], F16, tag="OUT")

            def fslot(m, il, imag):
                o = (FOFF[m] + (imag * RL[m] + (il - ILMIN[m]))) * NV
                return fF[:, o:o + NV]

            def fr(m, il):
                return fslot(m, il, 0)

            def fi(m, il):
                return fslot(m, il, 1)

            def pr(j):
                return P[:, j * NV:(j + 1) * NV]

            def pi(j):
                return P[:, (6 + j) * NV:(7 + j) * NV]

            def W(j):
                return fW[:, j * 128:(j + 1) * 128]

            def outr(m, il):
                o = (RO[m] + (il - ILMIN[m])) * NV
                return OUT[:, o:o + NV]

            def sc(col):
                return scal[:, col:col + 1]

            gr = G[:, 0:4 * NV]
            gi = G[:, 4 * NV:8 * NV]
            d0i = fD0[:, 0:4 * NV]

            def d0r(il):
                return fD0[:, (4 + il - 1) * NV:(4 + il) * NV]

            # ---- per-engine explicit chains: Tile's per-engine scheduler
            # reorders by readiness; pin the arrival-priority order.
            from bass_rust import add_dep_helper
            _prev = {}

            def chain(eng, ins):
                if eng in _prev:
                    add_dep_helper(ins.ins, _prev[eng].ins,
                                   reason=f"{eng} priority order")
                _prev[eng] = ins
                return ins

            # ---- input DMAs (sync HWDGE ring, strict arrival order)
            # early inputs ride the GpSimd SWDGE queue -- that engine
            # clears the entry preamble ~4us before the sync sequencer.
            def in_dma(eng, dst, o_nv, n_nv):
                eng.dma_start(dst, pin[:, o_nv * NV:(o_nv + n_nv) * NV])

            nc.gpsimd.dma_start(scal[:], pscal[:])
            for m in (1, 0):
                o = FOFF[m]
                in_dma(nc.gpsimd, fF[:, o * NV:(o + 2 * RL[m]) * NV],
                       o, 2 * RL[m])
            o = FOFF[2]
            in_dma(nc.sync, fF[:, o * NV:(o + 2 * RL[2]) * NV], o, 2 * RL[2])
            in_dma(nc.sync, fD0[:, 0:4 * NV], D0I_OFF, 4)      # D0i
            for m in (3, 4):
                o = FOFF[m]
                in_dma(nc.sync, fF[:, o * NV:(o + 2 * RL[m]) * NV],
                       o, 2 * RL[m])
            in_dma(nc.sync, fD0[:, 4 * NV:8 * NV], D0R_OFF, 4)  # D0r

            # ---- warm-gate memsets (DVE), identity (gpsimd), 12 diag weights
            chain("v", nc.vector.memset(ones128[:], 1.0))
            chain("v", nc.vector.memset(wrhs[:], 0.0))
            chain("g", nc.gpsimd.affine_select(
                out=W(DG_ONES), in_=ones128[:], pattern=[[1, 128]],
                compare_op=mybir.AluOpType.is_equal, fill=0.0, base=0,
                channel_multiplier=-1))
            for j in range(12):
                chain("v", nc.vector.tensor_scalar(
                    W(j), W(DG_ONES), sc(j), None, MULT))

            # ---- DVE prescales ----
            def presc_G(il):        # (Gr,Gi) = cC(il) * (Fr1,Fi1)
                chain("v", nc.vector.tensor_scalar_mul(
                    _pair(G[:, (il - 1) * NV:il * NV], 4 * NV),
                    _pair(fr(1, il), RL[1] * NV),
                    float(-il * (il + 1))))

            def presc(j, tt_eng="g"):  # P = 2cB*F[im+1] ; P += F[im-1]
                il, im = CB_PAIRS[j]
                chain("v", nc.vector.tensor_scalar_mul(
                    _pair(pr(j), 6 * NV),
                    _pair(fr(im + 1, il), RL[im + 1] * NV),
                    2.0 * _cB(il, im)))
                chain("v", nc.vector.tensor_tensor(
                    _pair(pr(j), 6 * NV),
                    _pair(pr(j), 6 * NV),
                    _pair(fr(im - 1, il), RL[im - 1] * NV),
                    ADD))

            # ---- PE program (emission = arrival order) ----
            def mm(bank, j, rhs, start=False, stop=False):
                chain("t", nc.tensor.matmul(bank, W(j), rhs, start=start,
                                            stop=stop,
                                            skip_group_check=True))

            def warm_mm(bank):
                chain("t", nc.tensor.matmul(bank, ones128[:], wrhs[:],
                                            start=True, stop=True,
                                            skip_group_check=True))

            # evacuation + output staging
            def evac(eng, pk, m, il):
                dst = _pair(outr(m, il), RL[m] * NV)
                if eng == "act":
                    chain("a", nc.scalar.copy(dst, pk[:]))
                else:
                    chain("v", nc.vector.tensor_copy(dst, pk[:]))

            def pair_tile(name):
                return ppool.tile([XS, 2 * NV], F32, tag="pk", name=name)

            def diag_chain(g, pk, order="ap_first"):
                bR, bI = pk[:, 0:NV], pk[:, NV:2 * NV]
                mp, il = g - 1, g
                if order == "ap_first":
                    mm(bR, DG_AP(g), fi(g, g), start=True)
                    mm(bI, DG_AN(g), fr(g, g), start=True)
                    mm(bR, DG_D1, fr(mp, il))
                    mm(bI, DG_D1, fi(mp, il))
                    mm(bR, DG_D3, fi(mp, il), stop=True)
                    mm(bI, DG_D2, fr(mp, il), stop=True)
                else:                      # d_first: AP/AN operands arrive last
                    mm(bR, DG_D1, fr(mp, il), start=True)
                    mm(bI, DG_D1, fi(mp, il), start=True)
                    mm(bR, DG_D3, fi(mp, il))
                    mm(bI, DG_D2, fr(mp, il))
                    mm(bR, DG_AP(g), fi(g, g), stop=True)
                    mm(bI, DG_AN(g), fr(g, g), stop=True)

            def mid_chain(il, g, pk):
                bR, bI = pk[:, 0:NV], pk[:, NV:2 * NV]
                j = CB_PAIRS.index((il, g))
                mm(bR, DG_AP(g), fi(g, il), start=True)
                mm(bI, DG_AN(g), fr(g, il), start=True)
                mm(bR, DG_D1, pr(j))
                mm(bI, DG_D1, pi(j))
                mm(bR, DG_D3, pi(j), stop=True)
                mm(bI, DG_D2, pr(j), stop=True)

            # PSUM: pairs rotate 2 bufs (4 banks); b0 pinned (4 banks)
            warm = pair_tile("warm")
            for k in range(NWARM):
                warm_mm(warm[:, (k % 2) * NV:(k % 2 + 1) * NV])

            b0p = [bpool.tile([XS, 2 * NV], F32, tag="b0", name="b0a"),
                   bpool.tile([XS, 2 * NV], F32, tag="b0", name="b0b")]

            def bk(il):
                return b0p[(il - 1) // 2][:, ((il - 1) % 2) * NV:
                                          ((il - 1) % 2 + 1) * NV]

            # --- run1 era: G, diag1 AP/AN + b0 partials
            for il in range(1, L1):
                presc_G(il)
            pk_d1 = pair_tile("d1")
            bR, bI = pk_d1[:, 0:NV], pk_d1[:, NV:2 * NV]
            mm(bR, DG_AP(1), fi(1, 1), start=True)
            mm(bI, DG_AN(1), fr(1, 1), start=True)
            for il in range(1, L1):
                mm(bk(il), DG_D6, G[:, (il - 1) * NV:il * NV], start=True)
            for il in range(1, L1):
                mm(bk(il), DG_D2, G[:, (3 + il) * NV:(4 + il) * NV])
            # --- run0: close diag1
            mm(bR, DG_D1, fr(0, 1))
            mm(bI, DG_D1, fi(0, 1))
            mm(bR, DG_D3, fi(0, 1), stop=True)
            mm(bI, DG_D2, fr(0, 1), stop=True)
            evac("act", pk_d1, 1, 1)
            # --- run2 era: diag2, g=1 mids
            pk_d2 = pair_tile("d2")
            diag_chain(2, pk_d2)
            evac("act", pk_d2, 2, 2)
            for j, (il, g) in ((0, (2, 1)), (1, (3, 1)), (2, (4, 1))):
                presc(j)
                pk = pair_tile(f"m{il}{g}")
                mid_chain(il, g, pk)
                evac("act", pk, g, il)
            # --- run3 era: diag3, g=2 mids
            pk_d3 = pair_tile("d3")
            diag_chain(3, pk_d3, order="d_first")
            evac("act", pk_d3, 3, 3)
            for j, (il, g) in ((3, (3, 2)), (4, (4, 2))):
                presc(j)
                pk = pair_tile(f"m{il}{g}")
                mid_chain(il, g, pk)
                evac("act", pk, g, il)
            # --- run4 era: m43 head, diag4, b0 close (D0r), m43 close
            presc(5, tt_eng="v")
            pk_m43 = pair_tile("m43")
            bR43, bI43 = pk_m43[:, 0:NV], pk_m43[:, NV:2 * NV]
            mm(bR43, DG_AP(3), fi(3, 4), start=True)
            mm(bI43, DG_AN(3), fr(3, 4), start=True)
            pk_d4 = pair_tile("d4")
            diag_chain(4, pk_d4, order="d_first")
            evac("dve", pk_d4, 4, 4)
            for il in range(1, L1):
                mm(bk(il), DG_ONES, d0r(il), stop=True)
            # Dr0 evacuation (frees the b0 banks for the Di0 phase)
            chain("a", nc.scalar.copy(
                OUT[:, RO0 * NV:(RO0 + 2) * NV], b0p[0][:]))
            chain("v", nc.vector.tensor_copy(
                OUT[:, (RO0 + 2) * NV:(RO0 + 4) * NV], b0p[1][:]))
            j43 = CB_PAIRS.index((4, 3))
            mm(bR43, DG_D1, pr(j43))
            mm(bI43, DG_D1, pi(j43))
            mm(bR43, DG_D3, pi(j43), stop=True)
            mm(bI43, DG_D2, pr(j43), stop=True)
            evac("dve", pk_m43, 3, 4)
            # Di0 phase: reuse b0 banks. Di0 = 0.5b1*Gi + 0.5b2*Gr + D0i
            def d0i_il(il):
                return fD0[:, (il - 1) * NV:il * NV]

            for il in range(1, L1):
                mm(bk(il), DG_D1, G[:, (3 + il) * NV:(4 + il) * NV],
                   start=True)
            for il in range(1, L1):
                mm(bk(il), DG_D2, G[:, (il - 1) * NV:il * NV])
            for il in range(1, L1):
                mm(bk(il), DG_ONES, d0i_il(il), stop=True)
            chain("a", nc.scalar.copy(
                OUT[:, (RO0 + 4) * NV:(RO0 + 6) * NV], b0p[0][:]))
            chain("v", nc.vector.tensor_copy(
                OUT[:, (RO0 + 6) * NV:(RO0 + 8) * NV], b0p[1][:]))

            # ---- output DMAs (sync ring, behind the inputs, completion
            # order; strided per-pair transfers start streaming early) ----
            def out_pair(m, il):
                o = (RO[m] + (il - ILMIN[m])) * NV
                nc.sync.dma_start(
                    _pair(pout[:, o:o + NV], RL[m] * NV),
                    _pair(outr(m, il), RL[m] * NV))

            def out_blk(o_nv, n_nv):
                nc.sync.dma_start(
                    pout[:, o_nv * NV:(o_nv + n_nv) * NV],
                    OUT[:, o_nv * NV:(o_nv + n_nv) * NV])

            def out_pair2(m, il):      # two adjacent pairs, one DMA
                o = (RO[m] + (il - ILMIN[m])) * NV
                nc.sync.dma_start(
                    _pair(pout[:, o:o + 2 * NV], RL[m] * NV),
                    _pair(OUT[:, o:o + 2 * NV], RL[m] * NV))

            out_pair(1, 1)             # d1
            out_pair(2, 2)             # d2
            out_pair(1, 2)             # m21
            out_pair2(1, 3)            # m31 + m41
            out_pair(3, 3)             # d3
            out_pair2(2, 3)            # m32 + m42
            out_pair(4, 4)             # d4
            out_blk(RO0, 4)            # Dr0
            out_pair(3, 4)             # m43
            out_blk(RO0 + 4, 4)        # Di0 (last)

    if split:
        split_multiwaits(nc)
    return nc


# ---------------------------------------------------------------------------
def pack_inputs(prev_f_re, prev_f_im, delta0_re, delta0_im, b):
    """-> list of per-core {'pin': [XS, CIN] f16, 'pscal': [XS, 16] f16}."""
    pr = np.asarray(prev_f_re, np.float32)
    pi = np.asarray(prev_f_im, np.float32)
    d0r = np.asarray(delta0_re, np.float32)
    d0i = np.asarray(delta0_im, np.float32)
    bb = np.asarray(b, np.float32)
    in_maps = []
    for c in range(NCORES):
        X = slice(c * XS, (c + 1) * XS)
        p = np.zeros((XS, CIN), np.float16)
        for m in range(L1):
            for il in range(ILMIN[m], L1):
                o = (FOFF[m] + (il - ILMIN[m])) * NV
                p[:, o:o + NV] = pr[il, m, X, :]
                o += RL[m] * NV
                p[:, o:o + NV] = pi[il, m, X, :]
        for il in range(1, L1):
            o = (D0I_OFF + il - 1) * NV
            p[:, o:o + NV] = d0i[il, 0, X, :]
            o = (D0R_OFF + il - 1) * NV
            p[:, o:o + NV] = d0r[il, 0, X, :]
        b0, b1, b2 = bb[X, 0], bb[X, 1], bb[X, 2]
        ps = np.zeros((XS, NSCAL), np.float32)
        cols = [0.5 * b1, 0.5 * b2, -0.5 * b2,
                1.0 * b0, 2.0 * b0, 3.0 * b0, 4.0 * b0,
                -1.0 * b0, -2.0 * b0, -3.0 * b0, -4.0 * b0,
                1.5 * b1]
        for j, cx in enumerate(cols):
            ps[:, j] = cx.astype(np.float16).astype(np.float32)
        in_maps.append({"pin": p, "pscal": ps})
    return in_maps


def unpack_outputs(results, delta0_re, delta0_im):
    out = np.zeros((L1, L1, NX, NV), np.complex64)
    out[0, 0] = np.asarray(delta0_re[0, 0]) + 1j * np.asarray(delta0_im[0, 0])
    for c in range(NCORES):
        X = slice(c * XS, (c + 1) * XS)
        p = results[c]["pout"]
        for m in range(1, L1):
            for il in range(m, L1):
                o = (RO[m] + (il - m)) * NV
                dr = p[:, o:o + NV].astype(np.float32)
                o += RL[m] * NV
                di = p[:, o:o + NV].astype(np.float32)
                out[il, m, X, :] = dr + 1j * di
        for il in range(1, L1):
            o = (RO0 + il - 1) * NV
            dr = p[:, o:o + NV].astype(np.float32)
            o = (RO0 + 4 + il - 1) * NV
            di = p[:, o:o + NV].astype(np.float32)
            out[il, 0, X, :] = dr + 1j * di
    return out


_NC_CACHE = None


def get_nc():
    global _NC_CACHE
    if _NC_CACHE is None:
        _NC_CACHE = build_bass()
    return _NC_CACHE


def kernel(prev_f_re, prev_f_im, delta0_re, delta0_im, b, v):
    in_maps = pack_inputs(prev_f_re, prev_f_im, delta0_re, delta0_im, b)
    res = run_bass_kernel_spmd(get_nc(), in_maps, list(range(NCORES)))
    return unpack_outputs(res.results, delta0_re, delta0_im)


# revision 12
# speedup vs baseline: 1.1538x; 1.1538x over previous
"""Trainium2 Bass kernel for nn_Bdfdv_51170240364850 (gnn_message_passing).

Computes, for mode pairs (il, im) with im <= il (L1 = 5 modes each way) and
spatial/velocity grid (nx=1024, nv=512):

  D[il,im] = base + (-1j)*im*bx*F[il,im] + cB*bm*F[il,im+1]
             + [im==0] Re(cC*bp*F[il,1])
  base     = 0.5*bm*F[il,im-1]  (il>=1, 1<=im<=il)   else  D0[il,im]

with bx = b[:,0], bm = b[:,1]+1j b[:,2], bp = conj(bm),
cB = -(il-im)(il+im+1)/2, cC = -il(il+1).

Strategy: pure data-parallel over nx across 8 NeuronCores (nx=128 per core on
the 128 SBUF partitions), fp16 I/O.  Every per-x product c(x)*T runs as a
diagonal-weight matmul accumulating in PSUM (diag(c) @ tile scales partition
row p by c(p)).  The 13 diagonal weight tiles are built ON-CHIP from a tiny
12-column scalar table (identity via affine_select, then one tensor_scalar
per diagonal), so the input stream is only the F/D0 payload (4.72 MB/core).

Pipeline: inputs stream in arrival order run1, run0, run2, run3, run4, D0i,
D0r on the sync HWDGE ring; the PE program is emitted in the same order so
each pair's 6-matmul chain closes right behind its data.  b0 (im=0) partial
sums live in a separate pinned PSUM pool so their long-lived banks (waiting
on the late D0r input) never block the rotating mid/diag pair banks --
the serialization that stalled the previous version.  Dense 512-wide warmup
matmuls at the head keep the HAM activity window busy so real chains run at
the warm 2.4 GHz clock.  Evacuations split ACT (early pairs) / DVE (late
pairs); outputs leave as 7 run-sized DMAs on the scalar HWDGE ring (early)
and the sync ring behind the inputs (late), so input+output interleave and
the 16 SDMA engines stay saturated.
"""

import numpy as np

import bass_rust
import concourse.bass as bass
import concourse.tile as tile
from concourse import mybir
from concourse.bass_utils import run_bass_kernel_spmd

L1 = 5
NX = 1024
NV = 512
NCORES = 8
XS = NX // NCORES  # 128, = SBUF partitions

F32 = mybir.dt.float32
F16 = mybir.dt.float16

# ---------------------------------------------------------------------------
# run/slot bookkeeping.  Run m holds slots (m, il) for il = ILMIN[m]..4,
# laid out [re slots | im slots] back-to-back so each run is ONE contiguous
# DMA.  Runs are stored in STREAM order run1, run0, run2, run3, run4.
RL = {0: 4, 1: 4, 2: 3, 3: 2, 4: 1}          # run lengths
ILMIN = {m: max(1, m) for m in range(L1)}
STREAM = [1, 0, 2, 3, 4]
FOFF = {}
_o = 0
for _m in STREAM:
    FOFF[_m] = _o
    _o += 2 * RL[_m]
assert _o == 28                               # NV-column units

# output block offsets (NV units): runs 1..4 then the im=0 block [Dr0|Di0]
RO = {1: 0, 2: 8, 3: 14, 4: 18}
RO0 = 20                                       # Dr0 at 20..24, Di0 at 24..28
COUT = 28 * NV

# input pin layout (NV units): F runs (28) | D0i (4) | D0r (4)
D0I_OFF = 28
D0R_OFF = 32
CIN = 36 * NV

# per-(il,im) recurrence pairs, in pipeline order (g=1 mids, then g=2, g=3)
CB_PAIRS = [(2, 1), (3, 1), (4, 1), (3, 2), (4, 2), (4, 3)]  # (il, im)


def _cB(il, im):
    return -(il - im) * (il + im + 1) / 2.0


# diagonal-weight table columns (pscal) and W tile indices
DG_D1, DG_D2, DG_D3 = 0, 1, 2                 # 0.5b1, 0.5b2, -0.5b2
DG_D6 = 11                                    # 1.5b1
DG_ONES = 12                                  # identity (built on-chip)
NSCAL = 16                                    # 12 used + padding


def DG_AP(m):
    return 2 + m          # 3..6:  +m*b0


def DG_AN(m):
    return 6 + m          # 7..10: -m*b0


# ---------------------------------------------------------------------------
# The walrus build in this container rejects instructions carrying more than
# ONE sync-wait ("Too many sync wait commands", setupSyncWait in
# CoreV2/V3GenImpl). Tile's scheduler routinely attaches several. Post-pass:
# hoist all but the last wait of each instruction onto same-engine NOPs
# inserted immediately before it (same basic block, so per-engine program
# order is preserved).
def split_multiwaits(nc):
    for f in nc.m.functions:
        for blk in f.blocks:
            new = []
            changed = False
            for ins in blk.instructions:
                si = ins.sync_info
                if si is not None and len(si.on_wait) > 1:
                    waits = list(si.on_wait)
                    for w in waits[:-1]:
                        nop = mybir.InstNoOp(
                            name=nc.get_next_instruction_name(),
                            engine=ins.engine,
                            bass_nofuse=True,
                            sync_info=mybir.SyncInfo(on_wait=[w],
                                                     on_update=[]),
                        )
                        new.append(nop)
                    ins.sync_info = bass_rust.SyncInfo(
                        on_wait=[waits[-1]], on_update=list(si.on_update))
                    changed = True
                new.append(ins)
            if changed:
                blk.instructions = new


# ---------------------------------------------------------------------------
def _pair(ap, step_elems, nblocks=2):
    """Turn a contiguous [P, L] AP into [P, nblocks, L] with the given
    element step between blocks."""
    c = ap.copy()
    v = c.ap
    last = v.pop()
    v.append((step_elems, nblocks))
    v.append(tuple(last))
    c.ap = v
    return c


NWARM = 10


def build_bass(split=True):
    MULT = mybir.AluOpType.mult
    ADD = mybir.AluOpType.add

    nc = bass.Bass()
    pin = nc.dram_tensor("pin", [XS, CIN], F16, kind="ExternalInput").ap()
    pscal = nc.dram_tensor("pscal", [XS, NSCAL], F32,
                           kind="ExternalInput").ap()
    pout = nc.dram_tensor("pout", [XS, COUT], F16, kind="ExternalOutput").ap()

    with tile.TileContext(nc) as tc:
        with tc.tile_pool(name="m", bufs=1) as pool, \
             tc.psum_pool(name="pp", bufs=2) as ppool, \
             tc.psum_pool(name="pb", bufs=2) as bpool:
            fF = pool.tile([XS, 28 * NV], F16, tag="fF")
            fD0 = pool.tile([XS, 8 * NV], F16, tag="fD0")
            scal = pool.tile([XS, NSCAL], F32, tag="scal")
            fW = pool.tile([XS, 13 * 128], F16, tag="fW")
            ones128 = pool.tile([XS, 128], F16, tag="ones")
            wrhs = pool.tile([XS, NV], F16, tag="wrhs")
            P = pool.tile([XS, 2 * 6 * NV], F16, tag="P")
            G = pool.tile([XS, 2 * 4 * NV], F16, tag="G")
            OUT = pool.tile([XS, 28 * NV], F16, tag="OUT")

            def fslot(m, il, imag):
                o = (FOFF[m] + (imag * RL[m] + (il - ILMIN[m]))) * NV
                return fF[:, o:o + NV]

            def fr(m, il):
                return fslot(m, il, 0)

            def fi(m, il):
                return fslot(m, il, 1)

            def pr(j):
                return P[:, j * NV:(j + 1) * NV]

            def pi(j):
                return P[:, (6 + j) * NV:(7 + j) * NV]

            def W(j):
                return fW[:, j * 128:(j + 1) * 128]

            def outr(m, il):
                o = (RO[m] + (il - ILMIN[m])) * NV
                return OUT[:, o:o + NV]

            def sc(col):
                return scal[:, col:col + 1]

            gr = G[:, 0:4 * NV]
            gi = G[:, 4 * NV:8 * NV]
            d0i = fD0[:, 0:4 * NV]

            def d0r(il):
                return fD0[:, (4 + il - 1) * NV:(4 + il) * NV]

            # ---- per-engine explicit chains: Tile's per-engine scheduler
            # reorders by readiness; pin the arrival-priority order.
            from bass_rust import add_dep_helper
            _prev = {}

            def chain(eng, ins):
                if eng in _prev:
                    add_dep_helper(ins.ins, _prev[eng].ins,
                                   reason=f"{eng} priority order")
                _prev[eng] = ins
                return ins

            # ---- input DMAs (sync HWDGE ring, strict arrival order).
            # run1/run0 are split so the (1,1)/(0,1) slots diag1 needs
            # arrive first and PE can start ~2.5us earlier.
            def in_dma(dst, src):
                nc.sync.dma_start(dst, src)

            in_dma(scal[:], pscal[:])
            in_dma(_pair(fF[:, 0:NV], 4 * NV),
                   _pair(pin[:, 0:NV], 4 * NV))                  # (1,1)
            in_dma(_pair(fF[:, NV:4 * NV], 4 * NV),
                   _pair(pin[:, NV:4 * NV], 4 * NV))             # run1 rest
            in_dma(_pair(fF[:, 8 * NV:9 * NV], 4 * NV),
                   _pair(pin[:, 8 * NV:9 * NV], 4 * NV))         # (0,1)
            in_dma(_pair(fF[:, 9 * NV:12 * NV], 4 * NV),
                   _pair(pin[:, 9 * NV:12 * NV], 4 * NV))        # run0 rest
            o = FOFF[2] * NV
            in_dma(fF[:, o:o + 6 * NV], pin[:, o:o + 6 * NV])    # run2
            in_dma(fD0[:, 0:4 * NV],
                   pin[:, D0I_OFF * NV:(D0I_OFF + 4) * NV])      # D0i
            o = FOFF[3] * NV
            in_dma(fF[:, o:o + 4 * NV], pin[:, o:o + 4 * NV])    # run3
            o = FOFF[4] * NV
            in_dma(fF[:, o:o + 2 * NV], pin[:, o:o + 2 * NV])    # run4
            in_dma(fD0[:, 4 * NV:8 * NV],
                   pin[:, D0R_OFF * NV:(D0R_OFF + 4) * NV])      # D0r

            # ---- warm-gate memsets (DVE), identity (gpsimd), 12 diag W
            chain("v", nc.vector.memset(ones128[:], 1.0))
            chain("v", nc.vector.memset(wrhs[:], 0.0))
            chain("g", nc.gpsimd.affine_select(
                out=W(DG_ONES), in_=ones128[:], pattern=[[1, 128]],
                compare_op=mybir.AluOpType.is_equal, fill=0.0, base=0,
                channel_multiplier=-1))
            for j in range(12):
                chain("v", nc.vector.tensor_scalar(
                    W(j), W(DG_ONES), sc(j), None, MULT))

            # ---- DVE prescales ----
            def presc_G(il):        # (Gr,Gi) = cC(il) * (Fr1,Fi1)
                chain("v", nc.vector.tensor_scalar_mul(
                    _pair(G[:, (il - 1) * NV:il * NV], 4 * NV),
                    _pair(fr(1, il), RL[1] * NV),
                    float(-il * (il + 1))))

            def presc(j):           # P = 2cB*F[im+1] ; P += F[im-1]
                il, im = CB_PAIRS[j]
                chain("v", nc.vector.tensor_scalar_mul(
                    _pair(pr(j), 6 * NV),
                    _pair(fr(im + 1, il), RL[im + 1] * NV),
                    2.0 * _cB(il, im)))
                chain("v", nc.vector.tensor_tensor(
                    _pair(pr(j), 6 * NV),
                    _pair(pr(j), 6 * NV),
                    _pair(fr(im - 1, il), RL[im - 1] * NV),
                    ADD))

            # ---- PE program (emission = arrival order) ----
            def mm(bank, j, rhs, start=False, stop=False):
                chain("t", nc.tensor.matmul(bank, W(j), rhs, start=start,
                                            stop=stop,
                                            skip_group_check=True))

            def warm_mm(bank):
                chain("t", nc.tensor.matmul(bank, ones128[:], wrhs[:],
                                            start=True, stop=True,
                                            skip_group_check=True))

            def evac(eng, pk, m, il):
                dst = _pair(outr(m, il), RL[m] * NV)
                if eng == "act":
                    chain("a", nc.scalar.copy(dst, pk[:]))
                else:
                    chain("v", nc.vector.tensor_copy(dst, pk[:]))

            def pair_tile(name):
                return ppool.tile([XS, 2 * NV], F32, tag="pk", name=name)

            def diag_chain(g, pk, order="ap_first"):
                bR, bI = pk[:, 0:NV], pk[:, NV:2 * NV]
                mp, il = g - 1, g
                if order == "ap_first":
                    mm(bR, DG_AP(g), fi(g, g), start=True)
                    mm(bI, DG_AN(g), fr(g, g), start=True)
                    mm(bR, DG_D1, fr(mp, il))
                    mm(bI, DG_D1, fi(mp, il))
                    mm(bR, DG_D3, fi(mp, il), stop=True)
                    mm(bI, DG_D2, fr(mp, il), stop=True)
                else:
                    mm(bR, DG_D1, fr(mp, il), start=True)
                    mm(bI, DG_D1, fi(mp, il), start=True)
                    mm(bR, DG_D3, fi(mp, il))
                    mm(bI, DG_D2, fr(mp, il))
                    mm(bR, DG_AP(g), fi(g, g), stop=True)
                    mm(bI, DG_AN(g), fr(g, g), stop=True)

            def mid_chain(il, g, pk):
                bR, bI = pk[:, 0:NV], pk[:, NV:2 * NV]
                j = CB_PAIRS.index((il, g))
                mm(bR, DG_AP(g), fi(g, il), start=True)
                mm(bI, DG_AN(g), fr(g, il), start=True)
                mm(bR, DG_D1, pr(j))
                mm(bI, DG_D1, pi(j))
                mm(bR, DG_D3, pi(j), stop=True)
                mm(bI, DG_D2, pr(j), stop=True)

            warm = pair_tile("warm")
            for k in range(NWARM):
                warm_mm(warm[:, (k % 2) * NV:(k % 2 + 1) * NV])

            b0p = [bpool.tile([XS, 2 * NV], F32, tag="b0", name="b0a"),
                   bpool.tile([XS, 2 * NV], F32, tag="b0", name="b0b")]

            def bk(il):
                return b0p[(il - 1) // 2][:, ((il - 1) % 2) * NV:
                                          ((il - 1) % 2 + 1) * NV]

            # --- diag1 (slot-split inputs: (1,1) then (0,1))
            pk_d1 = pair_tile("d1")
            bR, bI = pk_d1[:, 0:NV], pk_d1[:, NV:2 * NV]
            mm(bR, DG_AP(1), fi(1, 1), start=True)
            mm(bI, DG_AN(1), fr(1, 1), start=True)
            mm(bR, DG_D1, fr(0, 1))
            mm(bI, DG_D1, fi(0, 1))
            mm(bR, DG_D3, fi(0, 1), stop=True)
            mm(bI, DG_D2, fr(0, 1), stop=True)
            evac("act", pk_d1, 1, 1)
            # --- run1-rest era: G, b0 partials
            for il in range(1, L1):
                presc_G(il)
            for il in range(1, L1):
                mm(bk(il), DG_D6, G[:, (il - 1) * NV:il * NV], start=True)
            for il in range(1, L1):
                mm(bk(il), DG_D2, G[:, (3 + il) * NV:(4 + il) * NV])
            # --- run2 era: diag2, g=1 mids
            pk_d2 = pair_tile("d2")
            diag_chain(2, pk_d2)
            evac("act", pk_d2, 2, 2)
            for j, (il, g) in ((0, (2, 1)), (1, (3, 1)), (2, (4, 1))):
                presc(j)
                pk = pair_tile(f"m{il}{g}")
                mid_chain(il, g, pk)
                evac("act", pk, g, il)
            # im=0 imaginary row, first half (D0i arrives after run2)
            outdi = OUT[:, (RO0 + 4) * NV:(RO0 + 8) * NV]
            chain("v", nc.vector.scalar_tensor_tensor(
                outdi, gi, sc(DG_D1), d0i, MULT, ADD))
            # --- run3 era: diag3, g=2 mids
            pk_d3 = pair_tile("d3")
            diag_chain(3, pk_d3, order="d_first")
            evac("act", pk_d3, 3, 3)
            for j, (il, g) in ((3, (3, 2)), (4, (4, 2))):
                presc(j)
                pk = pair_tile(f"m{il}{g}")
                mid_chain(il, g, pk)
                evac("act", pk, g, il)
            # --- run4 era: m43 head, diag4, b0 close (D0r), m43 close
            presc(5)
            pk_m43 = pair_tile("m43")
            bR43, bI43 = pk_m43[:, 0:NV], pk_m43[:, NV:2 * NV]
            mm(bR43, DG_AP(3), fi(3, 4), start=True)
            mm(bI43, DG_AN(3), fr(3, 4), start=True)
            pk_d4 = pair_tile("d4")
            diag_chain(4, pk_d4, order="d_first")
            for il in range(1, L1):
                mm(bk(il), DG_ONES, d0r(il), stop=True)
            j43 = CB_PAIRS.index((4, 3))
            mm(bR43, DG_D1, pr(j43))
            mm(bI43, DG_D1, pi(j43))
            mm(bR43, DG_D3, pi(j43), stop=True)
            mm(bI43, DG_D2, pr(j43), stop=True)
            # im=0 imaginary row, second half; then tail evacs
            chain("v", nc.vector.scalar_tensor_tensor(
                outdi, gr, sc(DG_D2), outdi, MULT, ADD))
            evac("dve", pk_d4, 4, 4)
            evac("dve", pk_m43, 3, 4)
            chain("a", nc.scalar.copy(
                OUT[:, RO0 * NV:(RO0 + 2) * NV], b0p[0][:]))
            chain("a", nc.scalar.copy(
                OUT[:, (RO0 + 2) * NV:(RO0 + 4) * NV], b0p[1][:]))

            # ---- output DMAs (sync ring, behind inputs, completion order)
            def out_pair(m, il, nslots=1):
                o = (RO[m] + (il - ILMIN[m])) * NV
                nc.sync.dma_start(
                    _pair(pout[:, o:o + nslots * NV], RL[m] * NV),
                    _pair(OUT[:, o:o + nslots * NV], RL[m] * NV))

            def out_blk(o_nv, n_nv):
                nc.sync.dma_start(
                    pout[:, o_nv * NV:(o_nv + n_nv) * NV],
                    OUT[:, o_nv * NV:(o_nv + n_nv) * NV])

            out_pair(1, 1)             # d1
            out_pair(2, 2)             # d2
            out_pair(1, 2)             # m21
            out_pair(1, 3, 2)          # m31 + m41
            out_pair(3, 3)             # d3
            out_pair(2, 3, 2)          # m32 + m42
            out_pair(4, 4)             # d4
            out_blk(RO0 + 4, 4)        # Di0
            out_pair(3, 4)             # m43
            out_blk(RO0, 4)            # Dr0 (last)

    if split:
        split_multiwaits(nc)
    return nc


# ---------------------------------------------------------------------------
def pack_inputs(prev_f_re, prev_f_im, delta0_re, delta0_im, b):
    """-> list of per-core {'pin': [XS, CIN] f16, 'pscal': [XS, 16] f16}."""
    pr = np.asarray(prev_f_re, np.float32)
    pi = np.asarray(prev_f_im, np.float32)
    d0r = np.asarray(delta0_re, np.float32)
    d0i = np.asarray(delta0_im, np.float32)
    bb = np.asarray(b, np.float32)
    in_maps = []
    for c in range(NCORES):
        X = slice(c * XS, (c + 1) * XS)
        p = np.zeros((XS, CIN), np.float16)
        for m in range(L1):
            for il in range(ILMIN[m], L1):
                o = (FOFF[m] + (il - ILMIN[m])) * NV
                p[:, o:o + NV] = pr[il, m, X, :]
                o += RL[m] * NV
                p[:, o:o + NV] = pi[il, m, X, :]
        for il in range(1, L1):
            o = (D0I_OFF + il - 1) * NV
            p[:, o:o + NV] = d0i[il, 0, X, :]
            o = (D0R_OFF + il - 1) * NV
            p[:, o:o + NV] = d0r[il, 0, X, :]
        b0, b1, b2 = bb[X, 0], bb[X, 1], bb[X, 2]
        ps = np.zeros((XS, NSCAL), np.float32)
        cols = [0.5 * b1, 0.5 * b2, -0.5 * b2,
                1.0 * b0, 2.0 * b0, 3.0 * b0, 4.0 * b0,
                -1.0 * b0, -2.0 * b0, -3.0 * b0, -4.0 * b0,
                1.5 * b1]
        for j, cx in enumerate(cols):
            ps[:, j] = cx.astype(np.float16).astype(np.float32)
        in_maps.append({"pin": p, "pscal": ps})
    return in_maps


def unpack_outputs(results, delta0_re, delta0_im):
    out = np.zeros((L1, L1, NX, NV), np.complex64)
    out[0, 0] = np.asarray(delta0_re[0, 0]) + 1j * np.asarray(delta0_im[0, 0])
    for c in range(NCORES):
        X = slice(c * XS, (c + 1) * XS)
        p = results[c]["pout"]
        for m in range(1, L1):
            for il in range(m, L1):
                o = (RO[m] + (il - m)) * NV
                dr = p[:, o:o + NV].astype(np.float32)
                o += RL[m] * NV
                di = p[:, o:o + NV].astype(np.float32)
                out[il, m, X, :] = dr + 1j * di
        for il in range(1, L1):
            o = (RO0 + il - 1) * NV
            dr = p[:, o:o + NV].astype(np.float32)
            o = (RO0 + 4 + il - 1) * NV
            di = p[:, o:o + NV].astype(np.float32)
            out[il, 0, X, :] = dr + 1j * di
    return out


_NC_CACHE = None


def get_nc():
    global _NC_CACHE
    if _NC_CACHE is None:
        _NC_CACHE = build_bass()
    return _NC_CACHE


def kernel(prev_f_re, prev_f_im, delta0_re, delta0_im, b, v):
    in_maps = pack_inputs(prev_f_re, prev_f_im, delta0_re, delta0_im, b)
    res = run_bass_kernel_spmd(get_nc(), in_maps, list(range(NCORES)))
    return unpack_outputs(res.results, delta0_re, delta0_im)
